# revision 55
# baseline (speedup 1.0000x reference)
"""GSMNet GNN message-passing layer on 8 Trainium2 NeuronCores.

Sharding: edges partitioned across cores BY DESTINATION NODE (core c owns
dst in [c*N/8, (c+1)*N/8)), each core's edges sorted by dst, so the
per-node aggregation is core-local; only the BatchNorm statistics vectors
are all-reduced.

Host prep (all O(E*H) data movement / linear reductions):
  - 3-neighbor sums of edge_nei_len/angle (the mean is linear, folds into
    the up1 weights), per-node projections Gf = (x@Wf1a)[dst]+(x@Wf1b)[src]
    and Gm likewise (per-node GEMMs are 16x cheaper than per-edge),
  - all five per-edge streams pre-transposed to feature-major bf16
    [H, E_pad] so the device DMAs matmul-ready rhs tiles directly,
  - linear-into-linear weight folds as in the reference MLP algebra.

Device pipeline (activation-table-set disciplined: phase A uses only
silu_and_others functions - Silu, Tanh (sigmoid via tanh), Relu, Copy,
Sin; batch Rsqrt between phases; phase B uses sigmoid_and_others):
  A1 per 512-edge tile: load efT/lsT/asT, u1 = silu(.), gate-logits ->
     tanh, y = ef_lin + sigma*upd (fused DVE evacs), y -> SBUF ybuf;
     per-edge LayerNorm stats accumulate across ALL tiles into one
     [NT, 512] PSUM pair via one-hot-column lhsT matmuls.
  LN batch: mean/var -> single Rsqrt activation for all tiles.
  A2 per tile: broadcast mean/rstd via one-hot matmul, eo = relu(LN(y)),
     h1f/h1m = silu(f1c@eo + G + b) (G added in fused scalar_tensor_tensor
     from PSUM), z = f2@h1f (raw, bias cancels in batch-norm), z written
     IN PLACE over ybuf slot (SBUF-resident), mb -> DRAM scratch; BN-int
     stats via activation accum_out + square reduce.
  AllReduce BN-int stats; phase B: score = sigmoid(Ai*z+Bi) straight from
     SBUF z, msg = env*score*mb, PE-transpose, one-hot scatter matmul into
     SBUF-resident agg; AllReduce BN-out stats; fused residual+BN+relu.
"""

import math

import ml_dtypes
import numpy as np

import bass_rust
import concourse.bass as bass
import concourse.mybir as mybir
import concourse.tile as tile
from concourse.bass_utils import run_bass_kernel_spmd
from concourse.vector_clock import ScopedClock

dt = mybir.dt
F32 = dt.float32
BF16 = dt.bfloat16
NBF = ml_dtypes.bfloat16
NF8 = ml_dtypes.float8_e4m3
ALU = mybir.AluOpType
PM = mybir.MatmulPerfMode
ACTF = mybir.ActivationFunctionType

NCORES = 8
H = 256
ETILE = 512
CUTOFF = 5.0

# ---------------------------------------------------------------------------
# Walrus in this container rejects instructions carrying several semaphore
# waits on the no-struct ctrl path (the TileContext tail drain).  Split the
# drain's waits across single-wait nops.
_PATCHED = False


def _patch_tile_drain():
    global _PATCHED
    if _PATCHED:
        return

    _orig_lower = tile.TileContext._lower_ordered_insts
    _skip_types = ("TileBranchInst", "BassTileLoopBlock")
    _ws_id = [0]

    def _split_lower(self, ordered):
        for bb_name, insts in list(ordered.items()):
            new = []
            for inst in insts:
                if type(inst).__name__ in _skip_types:
                    new.append(inst)
                    continue
                try:
                    si = inst.sync_info
                    waits = list(si.on_wait) if si is not None else []
                except Exception:
                    waits = []
                if len(waits) > 1:
                    for w in waits[:-1]:
                        ev = bass_rust.InstEventSemaphore(
                            name=f"WS-{_ws_id[0]}")
                        _ws_id[0] += 1
                        ev.engine = inst.engine
                        ev.sync_info = bass_rust.SyncInfo(
                            on_wait=[w], on_update=[])
                        new.append(ev)
                    inst.sync_info = bass_rust.SyncInfo(
                        on_wait=[waits[-1]], on_update=list(si.on_update))
                new.append(inst)
            ordered[bb_name] = new
        return _orig_lower(self, ordered)

    tile.TileContext._lower_ordered_insts = _split_lower

    def _drain_and_barrier(self, tick_clock, wait_clock):
        probe = self.nc.sync.nop(nofuse=True)
        wait_clock.add_sem_waits(
            probe.ins, ScopedClock({None: tick_clock.global_clock})
        )
        waits = list(probe.ins.sync_info.on_wait)
        probe.ins.sync_info = bass_rust.SyncInfo(on_wait=waits[:1], on_update=[])
        for w in waits[1:]:
            inst = self.nc.sync.nop(nofuse=True)
            inst.ins.sync_info = bass_rust.SyncInfo(on_wait=[w], on_update=[])
        self.nc.sync.drain()
        self.nc.all_engine_barrier()
        popped = self.nc._tile_sem_poison_stack.pop()
        assert popped is self._sem_poison
        self.nc.clear_and_free_semaphores(list(self.sems.allocated().values()))
        self.nc.all_engine_barrier()

    tile.TileContext._drain_and_barrier = _drain_and_barrier
    _PATCHED = True


# ---------------------------------------------------------------------------
# host-side numerics helpers

WEIGHT_NAMES = [
    "u1f", "u1l", "u1a", "we", "w2", "gf", "gu", "f1c", "m1c", "f2", "m2",
]
FP8_WEIGHTS = {"u1f", "u1l", "u1a", "gf", "gu", "w2"}
BIAS_ORDER = [
    "u1b", "be", "b2", "gbh", "bf1", "bm1", "bm2",
    "lng", "lnb", "bnig", "bnib", "bnog", "bnob", "b2h",
]


def _bfr(a):
    # bf16 round-trip in float64 (matches device operand rounding)
    return np.asarray(a, np.float32).astype(NBF).astype(np.float64)


def _pack_w(w):
    # [K, M] -> [128, K//128, M] lhsT-chunk layout, bf16
    K, M = w.shape
    assert K % 128 == 0
    return np.ascontiguousarray(
        w.reshape(K // 128, 128, M).transpose(1, 0, 2)
    ).astype(NBF)


def _pack_b(b):
    # [256] -> [128, 2] per-partition chunks, fp32
    return np.ascontiguousarray(np.asarray(b).reshape(2, 128).T).astype(
        np.float32)


def _fold_weights(ins):
    g = lambda k: np.asarray(ins[k], np.float64)
    We, be = g("eu_lin_edge_w"), g("eu_lin_edge_b")
    Wl, bl = g("eu_lin_len_w"), g("eu_lin_len_b")
    Wa, ba = g("eu_lin_ang_w"), g("eu_lin_ang_b")
    W1, b1 = g("eu_up1_w"), g("eu_up1_b")
    W2, b2 = g("eu_up2_w"), g("eu_up2_b")
    Wg, bg = g("eu_gate_w"), g("eu_gate_b")
    Wf1, bf1 = g("mp_full1_w"), g("mp_full1_b")
    Wm1, bm1 = g("mp_msg1_w"), g("mp_msg1_b")
    Wm2, bm2 = g("mp_msg2_w"), g("mp_msg2_b")

    W1a, W1b, W1c = W1[0:H], W1[H : 2 * H], W1[2 * H : 3 * H]
    Wga, Wgb = Wg[0:H], Wg[H : 2 * H]
    weights = {
        "u1f": We @ W1a,
        "u1l": (Wl @ W1b) / 3.0,
        "u1a": (Wa @ W1c) / 3.0,
        "we": We,
        "w2": W2,
        "gf": We @ Wga,
        "gu": W2 @ Wgb,
        "f1c": Wf1[2 * H : 3 * H],
        "m1c": Wm1[2 * H : 3 * H],
        "f2": g("mp_full2_w"),
        "m2": Wm2,
    }
    gb_full = bg + be @ Wga + b2 @ Wgb
    biases = {
        "u1b": b1 + be @ W1a + bl @ W1b + ba @ W1c,
        "be": be,
        "b2": b2,
        "gbh": 0.5 * gb_full,
        "bf1": bf1,
        "bm1": bm1,
        "bm2": bm2,
        "lng": g("eu_ln_g"),
        "lnb": g("eu_ln_b"),
        "bnig": g("bn_int_g"),
        "bnib": g("bn_int_b"),
        "bnog": g("bn_out_g"),
        "bnob": g("bn_out_b"),
        "b2h": 0.5 * b2,
    }
    # per-node projection weights (host-side GEMMs)
    proj = {
        "pf1": Wf1[0:H], "pf2": Wf1[H : 2 * H],
        "pm1": Wm1[0:H], "pm2": Wm1[H : 2 * H],
    }
    return weights, biases, gb_full, proj


def _pad_edge_z(weights, biases, gb_full):
    """Host estimate of the RAW z (no bf2 bias) a zero-input pad edge
    produces on device, for BN-int stat correction."""
    sig = lambda v: 1.0 / (1.0 + np.exp(-v))
    f8r = lambda a: np.asarray(a, np.float32).astype(NF8).astype(np.float64)
    u1 = biases["u1b"].copy()
    u1s = f8r(u1 * sig(u1))
    pg = u1s @ f8r(weights["gu"]) + gb_full
    upd = u1s @ f8r(weights["w2"]) + biases["b2"]
    y = _bfr(biases["be"] + sig(pg) * upd)
    m, v = y.mean(), y.var()
    eo = np.maximum(
        (y - m) / np.sqrt(v + 1e-5) * biases["lng"] + biases["lnb"], 0.0
    )
    eo = _bfr(eo)
    h1 = eo @ _bfr(weights["f1c"]) + biases["bf1"]
    h1s = _bfr(h1 * sig(h1))
    z = _bfr(h1s @ _bfr(weights["f2"]))
    return z


def _cols(a, NT):
    # [E_pad] -> [128, NT*4]: edge (t,s,p) at [p, t*4+s]
    return np.ascontiguousarray(
        np.asarray(a, np.float32).reshape(NT * 4, 128).T
    )


def _fmT(a):
    # [E_pad, H] -> feature-major bf16 [H, E_pad]
    return np.ascontiguousarray(np.asarray(a, np.float32).T).astype(NBF)


def _prepare(inputs):
    x = np.asarray(inputs["x"], np.float32)
    ei = np.asarray(inputs["edge_index"])
    ef = np.asarray(inputs["edge_features"], np.float32)
    enl = np.asarray(inputs["edge_nei_len"], np.float32)
    ena = np.asarray(inputs["edge_nei_angle"], np.float32)
    el = np.asarray(inputs["edge_length"], np.float32)

    N, Hx = x.shape
    assert Hx == H
    E = ef.shape[0]
    assert N % NCORES == 0
    NLOC = N // NCORES

    # global linear reductions (fold into device matmuls)
    ls = enl.sum(1)                      # [E,H] 3-neighbor sums
    as_ = ena.sum(1)

    weights, biases, gb_full, proj = _fold_weights(inputs)
    Pf1 = x @ proj["pf1"].astype(np.float32)
    Pf2 = x @ proj["pf2"].astype(np.float32)
    Pm1 = x @ proj["pm1"].astype(np.float32)
    Pm2 = x @ proj["pm2"].astype(np.float32)

    src = np.asarray(ei[0], np.int64)
    dst = np.asarray(ei[1], np.int64)
    core_of = dst // NLOC

    perms, counts = [], []
    for c in range(NCORES):
        ids = np.nonzero(core_of == c)[0]
        order = np.argsort(dst[ids], kind="stable")
        perms.append(ids[order])
        counts.append(len(ids))
    NT = max(1, -(-max(counts) // ETILE))
    E_pad = NT * ETILE

    # static per-tile scatter-window bases shared across cores
    INF = 1 << 30
    lo = np.full((NCORES, NT), INF, np.int64)
    hi = np.full((NCORES, NT), -1, np.int64)
    for c in range(NCORES):
        dl = dst[perms[c]] - c * NLOC
        for t in range(NT):
            seg = dl[t * ETILE : (t + 1) * ETILE]
            if len(seg):
                lo[c, t] = seg[0]
                hi[c, t] = seg[-1]
    lo_t = lo.min(axis=0)
    hi_t = hi.max(axis=0)
    W = 128
    while True:
        base = np.minimum(np.where(lo_t == INF, 0, lo_t), max(NLOC - W, 0))
        if np.all(hi_t < base + W):
            break
        if W >= min(512, NLOC):
            raise RuntimeError("scatter window overflow")
        W = min(W * 2, 512, NLOC)
    base = base.astype(np.int64)

    z_pad = _pad_edge_z(weights, biases, gb_full)
    zp = _pack_b(z_pad)
    zp2 = _pack_b(z_pad * z_pad)

    wmaps = {}
    for k, v in weights.items():
        pw = _pack_w(_bfr(v))
        wmaps[f"w_{k}"] = pw.astype(NF8) if k in FP8_WEIGHTS else pw
    bias_arr = np.concatenate([_pack_b(biases[k]) for k in BIAS_ORDER], axis=1)

    iota = np.tile(np.arange(W, dtype=np.float32), (128, 1))
    ident = np.eye(128, dtype=np.float32).astype(NBF)
    sel = np.zeros((128, 2 * NT), np.float32)
    sel[:, NT - 1] = 1.0 / H
    sel = sel.astype(NBF)
    bsall = np.zeros((NT, NT * 128), np.float32)
    for t in range(NT):
        bsall[t, t * 128 : (t + 1) * 128] = 1.0
    bsall = bsall.astype(NBF)

    in_maps = []
    for c in range(NCORES):
        p = perms[c]
        cnt = counts[c]
        n_pad = E_pad - cnt

        def padT(src_arr, f8=False):
            a = np.zeros((E_pad, H), np.float32)
            a[:cnt] = src_arr[p]
            fm = _fmT(a)
            return fm.astype(NF8) if f8 else fm

        el_p = np.full(E_pad, 1e9, np.float32)
        el_p[:cnt] = el[p]
        src_p = np.zeros(E_pad, np.int64)
        src_p[:cnt] = src[p]
        dst_p = np.zeros(E_pad, np.int64)
        dst_p[:cnt] = dst[p]

        gf_p = np.zeros((E_pad, H), np.float32)
        gf_p[:cnt] = Pf1[dst_p[:cnt]] + Pf2[src_p[:cnt]]
        gm_p = np.zeros((E_pad, H), np.float32)
        gm_p[:cnt] = Pm1[dst_p[:cnt]] + Pm2[src_p[:cnt]]

        dl = dst_p - c * NLOC
        tile_of = np.arange(E_pad) // ETILE
        drel = dl - base[tile_of]
        drel[cnt:] = 0
        assert drel.min() >= 0 and drel.max() < W

        m = {
            "efT_in": padT(ef),
            "ef8T_in": padT(ef, f8=True),
            "lsT_in": padT(ls, f8=True),
            "asT_in": padT(as_, f8=True),
            "gfT_in": _fmT(gf_p),
            "gmT_in": _fmT(gm_p),
            "xT_loc": np.ascontiguousarray(x[c * NLOC : (c + 1) * NLOC].T),
            "len_cols": _cols(el_p, NT),
            "drel_cols": _cols(drel, NT),
            "corr": np.concatenate([zp, zp2], axis=1) * np.float32(n_pad),
            "biases": bias_arr.astype(np.float32),
            "iota": iota,
            "ident": ident,
            "sel": sel,
            "bsall": bsall,
        }
        m.update(wmaps)
        in_maps.append(m)

    ln_triv = bool(np.all(np.asarray(inputs["eu_ln_g"]) == 1.0)
                   and np.all(np.asarray(inputs["eu_ln_b"]) == 0.0))
    cfg = dict(N=N, NLOC=NLOC, E=E, E_pad=E_pad, NT=NT, W=W, ln_triv=ln_triv,
               base=tuple(int(b) for b in base))
    return cfg, in_maps


# ---------------------------------------------------------------------------
# device program


def _build_program(cfg):
    _patch_tile_drain()
    N, NLOC, E, E_pad, NT, W = (
        cfg["N"], cfg["NLOC"], cfg["E"], cfg["E_pad"], cfg["NT"], cfg["W"]
    )
    base = cfg["base"]

    nc = bass.Bass("TRN2", target_bir_lowering=False, debug=False,
                   num_devices=NCORES)

    FP8 = dt.float8e4
    efT_d = nc.dram_tensor("efT_in", [H, E_pad], BF16, kind="ExternalInput")
    ef8T_d = nc.dram_tensor("ef8T_in", [H, E_pad], FP8,
                            kind="ExternalInput")
    lsT_d = nc.dram_tensor("lsT_in", [H, E_pad], FP8, kind="ExternalInput")
    asT_d = nc.dram_tensor("asT_in", [H, E_pad], FP8, kind="ExternalInput")
    gfT_d = nc.dram_tensor("gfT_in", [H, E_pad], BF16, kind="ExternalInput")
    gmT_d = nc.dram_tensor("gmT_in", [H, E_pad], BF16, kind="ExternalInput")
    xT_d = nc.dram_tensor("xT_loc", [H, NLOC], F32, kind="ExternalInput")
    lenc_d = nc.dram_tensor("len_cols", [128, NT * 4], F32,
                            kind="ExternalInput")
    drel_d = nc.dram_tensor("drel_cols", [128, NT * 4], F32,
                            kind="ExternalInput")
    corr_d = nc.dram_tensor("corr", [128, 4], F32, kind="ExternalInput")
    bias_d = nc.dram_tensor("biases", [128, 2 * len(BIAS_ORDER)], F32,
                            kind="ExternalInput")
    iota_d = nc.dram_tensor("iota", [128, W], F32, kind="ExternalInput")
    ident_d = nc.dram_tensor("ident", [128, 128], BF16, kind="ExternalInput")
    sel_d = nc.dram_tensor("sel", [128, 2 * NT], BF16, kind="ExternalInput")
    bsall_d = nc.dram_tensor("bsall", [NT, NT * 128], BF16,
                             kind="ExternalInput")
    w_d = {k: nc.dram_tensor(f"w_{k}", [128, 2, H],
                             FP8 if k in FP8_WEIGHTS else BF16,
                             kind="ExternalInput")
           for k in WEIGHT_NAMES}

    out_d = nc.dram_tensor("out", [H, NLOC], F32, kind="ExternalOutput")

    ccA_in = nc.dram_tensor("ccA_in", [128, 4], F32)
    ccA_out = nc.dram_tensor("ccA_out", [128, 4], F32, addr_space="Shared")
    ccB_in = nc.dram_tensor("ccB_in", [128, 4], F32)
    ccB_out = nc.dram_tensor("ccB_out", [128, 4], F32, addr_space="Shared")

    RG = [list(range(NCORES))]

    with tile.TileContext(nc) as tc:
        with (
            tc.tile_pool(name="const", bufs=1) as cp,
            tc.tile_pool(name="io", bufs=2) as io,
            tc.tile_pool(name="wk", bufs=2) as wk,
            tc.tile_pool(name="ps", bufs=4, space="PSUM") as ps,
            tc.tile_pool(name="md", bufs=NT, space="DRAM") as mdp,
        ):
            # ---- resident constants
            wt = {}
            for k in WEIGHT_NAMES:
                t = cp.tile([128, 2, H], FP8 if k in FP8_WEIGHTS else BF16,
                            name=f"wt_{k}")
                nc.sync.dma_start(t[:], w_d[k][:])
                wt[k] = t
            bias_t = cp.tile([128, 2 * len(BIAS_ORDER)], F32)
            nc.sync.dma_start(bias_t[:], bias_d[:])

            def B(name):
                i = BIAS_ORDER.index(name)
                return bias_t[:, 2 * i : 2 * i + 2]

            iota_t = cp.tile([128, W], F32)
            nc.sync.dma_start(iota_t[:], iota_d[:])
            ident_t = cp.tile([128, 128], BF16)
            nc.sync.dma_start(ident_t[:], ident_d[:])
            sel_t = cp.tile([128, 2 * NT], BF16)
            nc.sync.dma_start(sel_t[:], sel_d[:])
            bsall_t = cp.tile([NT, NT * 128], BF16)
            nc.sync.dma_start(bsall_t[:], bsall_d[:])
            lenc_t = cp.tile([128, NT * 4], F32)
            nc.sync.dma_start(lenc_t[:], lenc_d[:])
            drel_t = cp.tile([128, NT * 4], F32)
            nc.sync.dma_start(drel_t[:], drel_d[:])
            corr_t = cp.tile([128, 4], F32)
            nc.sync.dma_start(corr_t[:], corr_d[:])
            halfpi_t = cp.tile([128, 1], F32)
            nc.vector.memset(halfpi_t[:], math.pi / 2)
            eps_t = cp.tile([128, 1], F32)
            nc.vector.memset(eps_t[:], 1e-5)

            # env = cos(min(len,5)*pi/10)^2, precomputed for all tiles
            # (Sin lives in silu_and_others: no extra table load)
            env_t = cp.tile([128, NT * 4], F32)
            nc.vector.tensor_scalar_min(env_t[:], lenc_t[:], CUTOFF)
            nc.scalar.activation(env_t[:], env_t[:], ACTF.Sin,
                                 bias=halfpi_t[:], scale=math.pi / 10)
            nc.vector.tensor_tensor(env_t[:], env_t[:], env_t[:], ALU.mult)

            agg = [cp.tile([128, NLOC], F32, name=f"agg{c}") for c in range(2)]
            nc.vector.memset(agg[0][:], 0.0)
            nc.vector.memset(agg[1][:], 0.0)


            # persistent per-tile y (phase A1) -> z (phase A2) buffer
            ybuf = cp.tile([128, NT, 2, ETILE], BF16, name="ybuf")
            # BN-int stat accumulators (per tile+chunk slots)
            zs_acc = cp.tile([128, 2, NT], F32)
            nc.vector.memset(zs_acc[:], 0.0)
            zq_acc = cp.tile([128, 2, NT], F32)

            def mm(psum, pairs, **kw):
                for i, (w, kc, mc, rhs) in enumerate(pairs):
                    nc.tensor.matmul(
                        psum[:], wt[w][:, kc, mc * 128 : (mc + 1) * 128],
                        rhs, start=(i == 0), stop=(i == len(pairs) - 1), **kw)

            def mmdr(psum, triples):
                # fp8 DoubleRow: both K-chunks of one H x H GEMM per instr
                for i, (w, mc, rhs) in enumerate(triples):
                    nc.tensor.matmul(
                        psum[:], wt[w][:, :, mc * 128 : (mc + 1) * 128],
                        rhs[:], start=(i == 0), stop=(i == len(triples) - 1),
                        perf_mode=PM.DoubleRow)

            mb_tiles = []

            # ======================= phase A1: y + LN stats ================
            with tc.tile_pool(name="stp", bufs=1, space="PSUM") as stp:
                pst_mean = stp.tile([NT, ETILE], F32, name="pst_mean")
                pst_m2 = stp.tile([NT, ETILE], F32, name="pst_m2")

                def emit_stats(pt, pysq):
                    # one-tile-delayed LN-stat matmuls (PE is in-order; y of
                    # tile pt is long done by now, so PE never stalls on DVE)
                    selt = sel_t[:, NT - 1 - pt : 2 * NT - 1 - pt]
                    for mc in range(2):
                        nc.tensor.matmul(
                            pst_mean[:], selt, ybuf[:, pt, mc, :],
                            start=(pt == 0 and mc == 0),
                            stop=(pt == NT - 1 and mc == 1),
                            skip_group_check=True)
                        nc.tensor.matmul(
                            pst_m2[:], selt, pysq[:, mc, :],
                            start=(pt == 0 and mc == 0),
                            stop=(pt == NT - 1 and mc == 1),
                            skip_group_check=True)

                prev_ysq = None
                for t in range(NT):
                    sl = slice(t * ETILE, (t + 1) * ETILE)
                    u1s = wk.tile([128, 2, ETILE], dt.float8e4, tag="u1s")
                    tg = wk.tile([128, 2, ETILE], BF16, tag="tg")
                    ysq = wk.tile([128, 2, ETILE], BF16, tag="ysq")
                    fT = io.tile([128, 2, ETILE], BF16, tag="fTa")
                    nc.sync.dma_start(
                        fT[:],
                        efT_d[:, sl].rearrange("(c p) e -> p c e", p=128))
                    f8T = io.tile([128, 2, ETILE], dt.float8e4, tag="f8T")
                    nc.sync.dma_start(
                        f8T[:],
                        ef8T_d[:, sl].rearrange("(c p) e -> p c e", p=128))
                    lT = io.tile([128, 2, ETILE], dt.float8e4, tag="lT8")
                    nc.sync.dma_start(
                        lT[:],
                        lsT_d[:, sl].rearrange("(c p) e -> p c e", p=128))
                    aT = io.tile([128, 2, ETILE], dt.float8e4, tag="aT8")
                    nc.sync.dma_start(
                        aT[:],
                        asT_d[:, sl].rearrange("(c p) e -> p c e", p=128))

                    for mc in range(2):
                        p = ps.tile([128, ETILE], F32, tag="mm")
                        mmdr(p, [("u1f", mc, f8T), ("u1l", mc, lT),
                                 ("u1a", mc, aT)])
                        nc.scalar.activation(
                            u1s[:, mc, :], p[:], ACTF.Silu,
                            bias=B("u1b")[:, mc : mc + 1])

                    for mc in range(2):
                        p_ef = ps.tile([128, ETILE], F32, tag="mm")
                        mm(p_ef, [("we", kc, mc, fT[:, kc, :])
                                  for kc in range(2)])
                        p = ps.tile([128, ETILE], F32, tag="mm")
                        mmdr(p, [("gf", mc, f8T), ("gu", mc, u1s)])
                        nc.scalar.activation(
                            tg[:, mc, :], p[:], ACTF.Tanh,
                            bias=B("gbh")[:, mc : mc + 1], scale=0.5)
                        p_up = ps.tile([128, ETILE], F32, tag="mm")
                        mmdr(p_up, [("w2", mc, u1s)])
                        u2 = wk.tile([128, ETILE], BF16, tag="u2")
                        nc.scalar.activation(
                            u2[:], p_up[:], ACTF.Identity, scale=0.5,
                            bias=B("b2h")[:, mc : mc + 1])
                        w_ = wk.tile([128, ETILE], BF16, tag="w_")
                        nc.vector.tensor_tensor(
                            w_[:], u2[:], tg[:, mc, :], ALU.mult)
                        y1 = wk.tile([128, ETILE], BF16, tag="y1")
                        nc.vector.scalar_tensor_tensor(
                            y1[:], p_ef[:], B("be")[:, mc : mc + 1], u2[:],
                            ALU.add, ALU.add)
                        yb = ybuf[:, t, mc, :]
                        nc.vector.tensor_tensor(yb, y1[:], w_[:], ALU.add)
                        nc.vector.tensor_tensor(
                            ysq[:, mc, :], yb, yb, ALU.mult)

                    if t > 0:
                        emit_stats(t - 1, prev_ysq)
                    prev_ysq = ysq
                emit_stats(NT - 1, prev_ysq)

                # ============= LN batch: mean/rstd for all tiles ===========
                # (one PSUM operand per DVE op: copy mean to SBUF f32 first)
                lnA = wk.tile([NT, ETILE], F32, tag="lnA", bufs=1)
                lnB = wk.tile([NT, ETILE], F32, tag="lnB", bufs=1)
                nc.vector.tensor_copy(lnA[:], pst_mean[:])
                mean_bf = cp.tile([NT, ETILE], BF16, name="mean_bf")
                nc.vector.tensor_copy(mean_bf[:], lnA[:])
                nc.vector.tensor_tensor(lnB[:], lnA[:], lnA[:], ALU.mult)
                nc.vector.tensor_tensor(lnB[:], pst_m2[:], lnB[:],
                                        ALU.subtract)
                nc.scalar.activation(lnA[:], lnB[:], ACTF.Sqrt,
                                     bias=eps_t[0:NT, :])
                nc.vector.reciprocal(lnB[:], lnA[:])
                rstd_bf = cp.tile([NT, ETILE], BF16, name="rstd_bf")
                nc.vector.tensor_copy(rstd_bf[:], lnB[:])

            # ================= phase A2: eo, h1, z, mb =====================
            with tc.tile_pool(name="bcp", bufs=2, space="PSUM") as bcp:
                def emit_bc(t):
                    bsel = bsall_t[:, t * 128 : (t + 1) * 128]
                    p_bc = bcp.tile([128, 2, ETILE], F32, tag="bc")
                    nc.tensor.matmul(p_bc[:, 0, :], bsel, mean_bf[:],
                                     start=True, stop=True)
                    nc.tensor.matmul(p_bc[:, 1, :], bsel, rstd_bf[:],
                                     start=True, stop=True)
                    return p_bc

                p_bc = emit_bc(0)
                for t in range(NT):
                    sl = slice(t * ETILE, (t + 1) * ETILE)
                    eoT = wk.tile([128, 2, ETILE], BF16, tag="eoT")
                    h1f = wk.tile([128, 2, ETILE], BF16, tag="h1f")
                    h1m = wk.tile([128, 2, ETILE], BF16, tag="h1m")
                    mbT = wk.tile([128, 2, ETILE], BF16, tag="mbT")
                    gfT = io.tile([128, 2, ETILE], BF16, tag="fT")
                    nc.sync.dma_start(
                        gfT[:],
                        gfT_d[:, sl].rearrange("(c p) e -> p c e", p=128))
                    gmT = io.tile([128, 2, ETILE], BF16, tag="lT")
                    nc.sync.dma_start(
                        gmT[:],
                        gmT_d[:, sl].rearrange("(c p) e -> p c e", p=128))

                    # mean/rstd broadcast -> SBUF bf16 (one DVE copy), then
                    # LN-apply on gpsimd, affine+relu on DVE tensor_scalar
                    bcsb = wk.tile([128, 2, ETILE], BF16, tag="bcsb")
                    nc.scalar.activation(bcsb[:], p_bc[:], ACTF.Copy)
                    for c in range(2):
                        t1 = wk.tile([128, ETILE], BF16, tag="t1")
                        nc.vector.tensor_tensor(
                            t1[:], ybuf[:, t, c, :], bcsb[:, 0, :],
                            ALU.subtract)
                        t2 = wk.tile([128, ETILE], BF16, tag="t2")
                        nc.vector.tensor_tensor(
                            t2[:], t1[:], bcsb[:, 1, :], ALU.mult)
                        if cfg.get("ln_triv"):
                            nc.vector.tensor_scalar_max(
                                eoT[:, c, :], t2[:], 0.0)
                        else:
                            t3 = wk.tile([128, ETILE], BF16, tag="t3")
                            nc.vector.tensor_scalar(
                                t3[:], t2[:], B("lng")[:, c : c + 1],
                                B("lnb")[:, c : c + 1], ALU.mult, ALU.add)
                            nc.vector.tensor_scalar_max(
                                eoT[:, c, :], t3[:], 0.0)

                    for mc in range(2):
                        p = ps.tile([128, ETILE], F32, tag="mm")
                        mm(p, [("f1c", kc, mc, eoT[:, kc, :])
                               for kc in range(2)])
                        nc.tensor.matmul(p[:], ident_t[:], gfT[:, mc, :],
                                         start=False, stop=True,
                                         skip_group_check=True)
                        nc.scalar.activation(
                            h1f[:, mc, :], p[:], ACTF.Silu,
                            bias=B("bf1")[:, mc : mc + 1])
                        p = ps.tile([128, ETILE], F32, tag="mm")
                        mm(p, [("m1c", kc, mc, eoT[:, kc, :])
                               for kc in range(2)])
                        nc.tensor.matmul(p[:], ident_t[:], gmT[:, mc, :],
                                         start=False, stop=True,
                                         skip_group_check=True)
                        nc.scalar.activation(
                            h1m[:, mc, :], p[:], ACTF.Silu,
                            bias=B("bm1")[:, mc : mc + 1])

                    if t + 1 < NT:
                        p_bc = emit_bc(t + 1)

                    for mc in range(2):
                        p = ps.tile([128, ETILE], F32, tag="mm")
                        mm(p, [("f2", kc, mc, h1f[:, kc, :])
                               for kc in range(2)])
                        # raw z (bias cancels in BN) written over ybuf slot
                        nc.scalar.activation(
                            ybuf[:, t, mc, :], p[:], ACTF.Copy,
                            accum_out=zs_acc[:, mc, t : t + 1])
                        zsq = wk.tile([128, ETILE], BF16, tag="zsq", bufs=1)
                        nc.vector.scalar_tensor_tensor(
                            zsq[:], ybuf[:, t, mc, :], 1.0,
                            ybuf[:, t, mc, :], ALU.mult, ALU.mult,
                            accum_out=zq_acc[:, mc, t : t + 1])
                        p = ps.tile([128, ETILE], F32, tag="mm")
                        mm(p, [("m2", kc, mc, h1m[:, kc, :])
                               for kc in range(2)])
                        nc.vector.tensor_scalar_add(
                            mbT[:, mc, :], p[:], B("bm2")[:, mc : mc + 1])

                    mb_dr = mdp.tile([128, 2 * ETILE], BF16, name=f"mb_dr{t}",
                                     tag=f"m{t}")
                    nc.sync.dma_start(
                        mb_dr[:], mbT[:].rearrange("p c e -> p (c e)"))
                    mb_tiles.append(mb_dr)

            # ============== BN-int stats allreduce -> Ai,Bi ==============
            zst = cp.tile([128, 4], F32)
            nc.vector.tensor_reduce(
                zst[:, 0:2], zs_acc[:], mybir.AxisListType.X, ALU.add)
            nc.vector.tensor_reduce(
                zst[:, 2:4], zq_acc[:], mybir.AxisListType.X, ALU.add)
            nc.vector.tensor_tensor(zst[:], zst[:], corr_t[:], ALU.subtract)
            nc.sync.dma_start(ccA_in[:], zst[:])
            nc.gpsimd.collective_compute(
                "AllReduce", ALU.add, ins=[ccA_in[:]], outs=[ccA_out[:]],
                replica_groups=RG)
            gA = cp.tile([128, 4], F32)
            nc.sync.dma_start(gA[:], ccA_out[:])
            mInt = cp.tile([128, 2], F32)
            nc.vector.tensor_scalar_mul(mInt[:], gA[:, 0:2], 1.0 / E)
            vInt = cp.tile([128, 2], F32)
            nc.vector.tensor_scalar_mul(vInt[:], gA[:, 2:4], 1.0 / E)
            msq = cp.tile([128, 2], F32)
            nc.vector.tensor_tensor(msq[:], mInt[:], mInt[:], ALU.mult)
            nc.vector.tensor_tensor(vInt[:], vInt[:], msq[:], ALU.subtract)
            nc.scalar.activation(vInt[:], vInt[:], ACTF.Sqrt, bias=eps_t[:])
            invI = cp.tile([128, 2], F32)
            nc.vector.reciprocal(invI[:], vInt[:])
            Ai = cp.tile([128, 2], F32)
            nc.vector.tensor_tensor(Ai[:], invI[:], B("bnig"), ALU.mult)
            Bi = cp.tile([128, 2], F32)
            nc.vector.tensor_tensor(Bi[:], mInt[:], Ai[:], ALU.mult)
            nc.vector.tensor_tensor(Bi[:], B("bnib"), Bi[:], ALU.subtract)

            # =========================== phase B ===========================
            bp_cm = tc.tile_pool(name="bp", bufs=2, space="PSUM")
            bp = bp_cm.__enter__()
            for t in range(NT):
                msgT = wk.tile([128, 2, ETILE], BF16, tag="msgT")
                mbL = io.tile([128, 2, ETILE], BF16, tag="aT")
                nc.sync.dma_start(
                    mbL[:], mb_tiles[t][:].rearrange("p (c e) -> p c e", c=2))

                for c in range(2):
                    sc = wk.tile([128, ETILE], BF16, tag="scB", bufs=1)
                    nc.scalar.activation(
                        sc[:], ybuf[:, t, c, :], ACTF.Sigmoid,
                        bias=Bi[:, c : c + 1], scale=Ai[:, c : c + 1])
                    nc.vector.tensor_tensor(
                        msgT[:, c, :], sc[:], mbL[:, c, :], ALU.mult)

                msg_em = wk.tile([128, 4, H], BF16, tag="msg_em", bufs=2)
                for s in range(4):
                    tp = bp.tile([128, ETILE], BF16, tag="tp")
                    for c in range(2):
                        nc.tensor.transpose(
                            tp[:, c * 128 : (c + 1) * 128],
                            msgT[:, c, s * 128 : (s + 1) * 128], ident_t[:])
                    if s % 2 == 0:
                        nc.vector.tensor_copy(msg_em[:, s, :], tp[:, 0:H])
                    else:
                        nc.scalar.activation(msg_em[:, s, :], tp[:, 0:H],
                                             ACTF.Copy)

                oh = wk.tile([128, 4, W], BF16, tag="oh", bufs=2)
                for s in range(4):
                    nc.vector.tensor_scalar(
                        oh[:, s, :], iota_t[:],
                        drel_t[:, 4 * t + s : 4 * t + s + 1],
                        env_t[:, 4 * t + s : 4 * t + s + 1],
                        ALU.is_equal, ALU.mult)
                b0 = base[t]
                for c in range(2):
                    p = bp.tile([128, W], F32, tag="sc", bufs=2)
                    for s in range(4):
                        nc.tensor.matmul(
                            p[:], msg_em[:, s, c * 128 : (c + 1) * 128],
                            oh[:, s, :], start=(s == 0), stop=(s == 3))
                    nc.vector.tensor_tensor(
                        agg[c][:, b0 : b0 + W], agg[c][:, b0 : b0 + W], p[:],
                        ALU.add)

            bp_cm.__exit__(None, None, None)

            # ============== BN-out stats allreduce + final ==============
            ast = cp.tile([128, 4], F32)
            scr2 = wk.tile([128, NLOC], F32, tag="xL", bufs=1)
            for c in range(2):
                nc.vector.tensor_reduce(
                    ast[:, c : c + 1], agg[c][:], mybir.AxisListType.X,
                    ALU.add)
                nc.vector.tensor_tensor(
                    scr2[:], agg[c][:], agg[c][:], ALU.mult)
                nc.vector.tensor_reduce(
                    ast[:, 2 + c : 3 + c], scr2[:],
                    mybir.AxisListType.X, ALU.add)
            nc.sync.dma_start(ccB_in[:], ast[:])
            nc.gpsimd.collective_compute(
                "AllReduce", ALU.add, ins=[ccB_in[:]], outs=[ccB_out[:]],
                replica_groups=RG)
            gB = cp.tile([128, 4], F32)
            nc.sync.dma_start(gB[:], ccB_out[:])
            mO = cp.tile([128, 2], F32)
            nc.vector.tensor_scalar_mul(mO[:], gB[:, 0:2], 1.0 / N)
            vO = cp.tile([128, 2], F32)
            nc.vector.tensor_scalar_mul(vO[:], gB[:, 2:4], 1.0 / N)
            msqO = cp.tile([128, 2], F32)
            nc.vector.tensor_tensor(msqO[:], mO[:], mO[:], ALU.mult)
            nc.vector.tensor_tensor(vO[:], vO[:], msqO[:], ALU.subtract)
            nc.scalar.activation(vO[:], vO[:], ACTF.Sqrt, bias=eps_t[:])
            invO = cp.tile([128, 2], F32)
            nc.vector.reciprocal(invO[:], vO[:])
            A2 = cp.tile([128, 2], F32)
            nc.vector.tensor_tensor(A2[:], invO[:], B("bnog"), ALU.mult)
            B2 = cp.tile([128, 2], F32)
            nc.vector.tensor_tensor(B2[:], mO[:], A2[:], ALU.mult)
            nc.vector.tensor_tensor(B2[:], B("bnob"), B2[:], ALU.subtract)

            for c in range(2):
                xL = wk.tile([128, NLOC], F32, tag="xL", bufs=1)
                nc.sync.dma_start(xL[:], xT_d[c * 128 : (c + 1) * 128, :])
                ot = wk.tile([128, NLOC], F32, tag="ot", bufs=1)
                nc.vector.tensor_scalar(
                    ot[:], agg[c][:], A2[:, c : c + 1], B2[:, c : c + 1],
                    ALU.mult, ALU.add)
                nc.vector.tensor_tensor(ot[:], ot[:], xL[:], ALU.add)
                nc.vector.tensor_scalar_max(ot[:], ot[:], 0.0)
                nc.sync.dma_start(out_d[c * 128 : (c + 1) * 128, :], ot[:])

    return nc


# ---------------------------------------------------------------------------

_CACHE = {}


def _get_program(cfg):
    key = tuple(sorted((k, v) for k, v in cfg.items()))
    if key not in _CACHE:
        _CACHE[key] = _build_program(cfg)
    return _CACHE[key]


def _assemble(cfg, results):
    N, NLOC = cfg["N"], cfg["NLOC"]
    out = np.empty((N, H), np.float32)
    for c in range(NCORES):
        out[c * NLOC : (c + 1) * NLOC] = results[c]["out"].T
    return out


def kernel(**inputs):
    cfg, in_maps = _prepare(inputs)
    nc = _get_program(cfg)
    res = run_bass_kernel_spmd(nc, in_maps, list(range(NCORES)))
    return _assemble(cfg, res.results)
